# revision 15
# baseline (speedup 1.0000x reference)
"""BitLinear (RMSNorm + per-tensor 8-bit act quant + ternary weight quant + matmul)
as a distributed Bass/Tile kernel on 8 TRN2 NeuronCores.

Sharding: data-parallel over tokens (B*S = 32768 -> 4096 tokens/core).
Every core loads the full (host-pre-transposed) weight and computes
w_scale redundantly; the only collective is an AllGather of per-core
(max|xn|, -1/max|xn|) pairs (gathering the negated inverse avoids any
post-collective reciprocal: max_r(-1/m_r) == -1/max_r m_r).

v5 pipeline (engine-disciplined, strict-FIFO-safe):
  * x cast to fp16 on host.  Tokens are interleaved 4-per-partition
    ("(p c) k" instead of "(c p) k") so every DMA partition line is a
    contiguous 8KB run -- the 2KB-line layout measured only ~178 GB/s.
    The permutation is carried consistently through stats, the fused
    transpose, the GEMM and the output DMA.
  * Phase A: sumsq on scalar (Square+accum; tensor_tensor_reduce
    crashes real HW), per-chunk absmax on DVE.  Collective launches as
    soon as stats finish (~40us).
  * rms fused into the PE transpose via matmul against diag(rms).
  * Weight DMA interleaved with x DMA; |w| accumulation on DVE; weight
    quantization scalar+DVE, in place.
  * gpsimd owns diag builds, the collective path, and the entire
    phase-B quantize (scale+magic-add, magic-sub) so scalar/DVE only
    carry the PSUM->SBUF copies once the GEMM is running.
  * Output fp16, host upcasts.

Numerical core: x_q in [-127,127] integers, w_q in {-1,0,1}; fp16
matmul with fp32 PSUM accumulation is exact; rounding via the fp32
magic constant (1.5*2**23) matches jnp.round.
"""

import numpy as np

# ---- problem constants (hardcoded per contract) ----
B, S, DIN, DOUT = 4, 8192, 1024, 1024
N_CORES = 8
TOK = B * S                    # 32768 tokens
TOK_C = TOK // N_CORES         # 4096 tokens per core
TPD = 512                      # tokens per DMA chunk (4 per partition)
ND = TOK_C // TPD              # 8 DMA chunks per core
SUB = TPD // 128               # 4 sub-tiles per chunk (token = 4p + c)
NT = TOK_C // 128              # 32 sub-tiles
KT = DIN // 128                # 8 contraction tiles
NH = DOUT // 512               # 2 psum halves of the output row
EPS = 1e-6
QP = 127.0
MAGIC = 12582912.0             # 1.5 * 2**23: fp32 RNE round-to-int trick

_CACHE = {}


def _build(apply_nw: bool):
    import concourse.bass as bass
    import concourse.bacc as bacc
    import concourse.mybir as mybir
    import concourse.bass_isa as bass_isa
    from concourse import tile, masks

    f32 = mybir.dt.float32
    fp16 = mybir.dt.float16
    AF = mybir.ActivationFunctionType
    OP = mybir.AluOpType
    AX = mybir.AxisListType
    RED = bass_isa.ReduceOp

    nc = bacc.Bacc("TRN2", target_bir_lowering=False, debug=False,
                   num_devices=N_CORES)

    x_d = nc.dram_tensor("x", [TOK_C, DIN], fp16, kind="ExternalInput")
    wt_d = nc.dram_tensor("wt", [DIN, DOUT], f32, kind="ExternalInput")
    if apply_nw:
        nw_d = nc.dram_tensor("nw", [128, DIN], fp16, kind="ExternalInput")
    out_d = nc.dram_tensor("out", [TOK_C, DOUT], fp16, kind="ExternalOutput")

    with tile.TileContext(nc) as tc:
        with (
            tc.tile_pool(name="const", bufs=1) as const_pool,
            tc.tile_pool(name="stats", bufs=1) as stats,
            tc.tile_pool(name="xs", bufs=ND) as x_pool,
            tc.tile_pool(name="xnT", bufs=NT) as xnT_pool,
            tc.tile_pool(name="wts", bufs=KT) as wt_pool,
            tc.tile_pool(name="wqs", bufs=KT) as wq_pool,
            tc.tile_pool(name="sqscr", bufs=2) as sq_pool,
            tc.tile_pool(name="diag", bufs=NT) as diag_pool,
            tc.tile_pool(name="dram", bufs=1, space="DRAM") as dram_pool,
        ):
            ident_hf = const_pool.tile([128, 128], fp16, tag="ident_hf")
            masks.make_identity(nc, ident_hf[:, :])

            sumsq = stats.tile([128, NT], f32, tag="sumsq")
            amax = stats.tile([128, NT], f32, tag="amax")
            rms = stats.tile([128, NT], f32, tag="rms")
            wsum = stats.tile([128, KT], f32, tag="wsum")

            xnT_list = []
            wq_list = []
            dg_tiles = []
            wt_tiles = []

            if apply_nw:
                nwb = const_pool.tile([128, DIN], fp16, tag="nwb")
                nc.sync.dma_start(out=nwb[:, :], in_=nw_d[:, :])

            # ---------- x load (8KB lines) + stats; wt DMA interleaved ------
            x_tiles = []
            for d in range(ND):
                xt = x_pool.tile([128, SUB, DIN], fp16, tag="xt")
                nc.sync.dma_start(
                    out=xt[:, :, :],
                    in_=x_d[d * TPD:(d + 1) * TPD, :].rearrange(
                        "(p c) k -> p c k", p=128))
                if d >= 1:   # interleave weight loads behind the x stream
                    j = d - 1
                    wtt = wt_pool.tile([128, DOUT], f32, tag="wt")
                    nc.sync.dma_start(out=wtt[:, :],
                                      in_=wt_d[j * 128:(j + 1) * 128, :])
                    wt_tiles.append(wtt)
                if apply_nw:
                    xh = x_pool.tile([128, SUB, DIN], fp16, tag="xh")
                    for c in range(SUB):
                        nc.vector.tensor_tensor(out=xh[:, c, :],
                                                in0=xt[:, c, :],
                                                in1=nwb[:, :], op=OP.mult)
                else:
                    xh = xt
                x_tiles.append(xh)
                for c in range(SUB):
                    i = d * SUB + c
                    scr = sq_pool.tile([128, DIN], fp16, tag="sqa")
                    nc.scalar.activation(
                        out=scr[:, :], in_=xt[:, c, :], func=AF.Square,
                        accum_out=sumsq[:, i:i + 1])
                # per-chunk absmax: [128, SUB, DIN] -> [128, SUB] in one op
                sl = slice(d * SUB, (d + 1) * SUB)
                nc.vector.tensor_reduce(
                    out=amax[:, sl], in_=xh[:, :, :],
                    axis=AX.X, op=OP.max, apply_absolute_value=True)
                m2 = stats.tile([128, SUB], f32, tag="m2", name=f"m2_{d}")
                nc.vector.tensor_scalar(out=m2[:, :], in0=sumsq[:, sl],
                                        scalar1=1.0 / DIN, scalar2=EPS,
                                        op0=OP.mult, op1=OP.add)
                r2 = stats.tile([128, SUB], f32, tag="r2", name=f"r2_{d}")
                nc.vector.reciprocal(r2[:, :], m2[:, :])
                nc.scalar.activation(out=rms[:, sl], in_=r2[:, :],
                                     func=AF.Sqrt)
                # diag(rms) builds on gpsimd (it idles until the collective)
                for c in range(SUB):
                    i = d * SUB + c
                    dg = diag_pool.tile([128, 128], fp16, tag="dg",
                                        name=f"dg_{i}")
                    nc.gpsimd.tensor_scalar(out=dg[:, :], in0=ident_hf[:, :],
                                            scalar1=rms[:, i:i + 1],
                                            scalar2=None, op0=OP.mult)
                    dg_tiles.append(dg)
            # last weight tile
            wtt = wt_pool.tile([128, DOUT], f32, tag="wt")
            nc.sync.dma_start(out=wtt[:, :], in_=wt_d[(KT - 1) * 128:, :])
            wt_tiles.append(wtt)

            # ---------- local |xn| max -> collective (no PE involved) --------
            axn = stats.tile([128, NT], f32, tag="axn")
            nc.vector.tensor_tensor(out=axn[:, :], in0=amax[:, :],
                                    in1=rms[:, :], op=OP.mult)
            axn2 = stats.tile([128, NT], f32, tag="axn2")
            nc.vector.tensor_scalar(out=axn2[:, :], in0=axn[:, :],
                                    scalar1=1e4, scalar2=None, op0=OP.min)
            lmax = stats.tile([128, 1], f32, tag="lmax")
            nc.vector.tensor_reduce(out=lmax[:, :], in_=axn2[:, :],
                                    axis=AX.X, op=OP.max)
            gmax = stats.tile([128, 1], f32, tag="gmax")
            nc.gpsimd.partition_all_reduce(gmax[:, :], lmax[:, :],
                                           channels=128, reduce_op=RED.max)
            ginv = stats.tile([128, 1], f32, tag="ginv")
            nc.vector.reciprocal(ginv[:, :], gmax[:, :])
            pk = stats.tile([1, 2], f32, tag="pk")
            nc.vector.tensor_copy(pk[:, 0:1], gmax[0:1, 0:1])
            nc.vector.tensor_scalar(out=pk[:, 1:2], in0=ginv[0:1, 0:1],
                                    scalar1=-1.0, scalar2=None, op0=OP.mult)

            cc_in = dram_pool.tile([1, 2], f32, tag="cc_in")
            cc_out = dram_pool.tile([1, 2 * N_CORES], f32, tag="cc_out")
            nc.sync.dma_start(out=cc_in[:, :], in_=pk[:, :])
            nc.gpsimd.collective_compute(
                "AllGather", OP.bypass,
                replica_groups=[list(range(N_CORES))],
                ins=[cc_in[:, :].opt()],
                outs=[cc_out[:, :].opt()],
            )

            # ---------- w_scale (|w| accumulation on DVE) --------------------
            for j in range(KT):
                nc.vector.tensor_reduce(
                    out=wsum[:, j:j + 1], in_=wt_tiles[j][:, :],
                    axis=AX.X, op=OP.add, apply_absolute_value=True)
            wred = stats.tile([128, 1], f32, tag="wred")
            nc.vector.tensor_reduce(out=wred[:, :], in_=wsum[:, :],
                                    axis=AX.X, op=OP.add)
            wrow_d = dram_pool.tile([1, 128], f32, tag="wrow_d")
            nc.sync.dma_start(out=wrow_d[:, :], in_=wred[:, :])
            wrow = stats.tile([1, 128], f32, tag="wrow")
            nc.sync.dma_start(out=wrow[:, :], in_=wrow_d[:, :])
            wtot = stats.tile([1, 1], f32, tag="wtot")
            nc.vector.tensor_reduce(out=wtot[:, :], in_=wrow[:, :],
                                    axis=AX.X, op=OP.add)
            wsc = stats.tile([1, 1], f32, tag="wsc")
            nc.vector.tensor_scalar(out=wsc[:, :], in0=wtot[:, :],
                                    scalar1=1.0 / (DIN * DOUT),
                                    scalar2=1e-4, op0=OP.mult, op1=OP.max)
            inv_ws = stats.tile([1, 1], f32, tag="inv_ws")
            nc.vector.reciprocal(inv_ws[:, :], wsc[:, :])
            wpk = stats.tile([1, 2], f32, tag="wpk")
            nc.vector.tensor_copy(wpk[:, 0:1], wsc[:, :])
            nc.vector.tensor_copy(wpk[:, 1:2], inv_ws[:, :])
            wb_d = dram_pool.tile([1, 2], f32, tag="wb_d")
            nc.sync.dma_start(out=wb_d[:, :], in_=wpk[:, :])
            wb2 = stats.tile([128, 2], f32, tag="wb2")
            nc.sync.dma_start(out=wb2[:, :],
                              in_=wb_d[0:1, :].broadcast_to([128, 2]))

            # weight quantization: scalar magic-round, DVE clip (in place)
            for j in range(KT):
                nc.scalar.activation(out=wt_tiles[j][:, :],
                                     in_=wt_tiles[j][:, :], func=AF.Copy,
                                     scale=wb2[:, 1:2], bias=MAGIC)
                nc.vector.tensor_scalar(out=wt_tiles[j][:, :],
                                        in0=wt_tiles[j][:, :],
                                        scalar1=MAGIC, scalar2=1.0,
                                        op0=OP.subtract, op1=OP.min)
                wq = wq_pool.tile([128, DOUT], fp16, tag="wq")
                nc.vector.tensor_scalar(out=wq[:, :], in0=wt_tiles[j][:, :],
                                        scalar1=-1.0, scalar2=None,
                                        op0=OP.max)
                wq_list.append(wq)

            # ---------- transposes: diag(rms)-fused, fill the cc bubble ------
            with tc.tile_pool(name="psA", bufs=3, space="PSUM") as psA:
                for i in range(NT):
                    d, c = divmod(i, SUB)
                    pA = psA.tile([128, DIN], f32, tag="pA")
                    for j in range(KT):
                        nc.tensor.matmul(
                            pA[:, j * 128:(j + 1) * 128],
                            lhsT=x_tiles[d][:, c, j * 128:(j + 1) * 128],
                            rhs=dg_tiles[i][:, :], start=True, stop=True)
                    xnT = xnT_pool.tile([128, DIN], fp16, tag="xnT",
                                        name=f"xnT_{i}")
                    xnT_list.append(xnT)
                    # psum f32 -> sbuf fp16: ~17 on ACT, ~15 on DVE
                    if i % 2 == 0 or i == 31:
                        nc.scalar.activation(out=xnT[:, :], in_=pA[:, :],
                                             func=AF.Copy)
                    else:
                        nc.vector.tensor_copy(xnT[:, :], pA[:, :])

            # ---------- collective read-back + scales (all gpsimd) -----------
            g_sb = stats.tile([N_CORES, 2], f32, tag="g_sb")
            nc.sync.dma_start(
                out=g_sb[:, :],
                in_=cc_out[0:1, :].rearrange("a (p c) -> (a p) c", p=N_CORES))
            g8 = stats.tile([N_CORES, 2], f32, tag="g8")
            nc.gpsimd.partition_all_reduce(g8[:, :], g_sb[:, :],
                                           channels=N_CORES,
                                           reduce_op=RED.max)
            gb = stats.tile([128, 2], f32, tag="gb")
            nc.gpsimd.partition_broadcast(gb[:, :], g8[0:1, :])
            a_b = stats.tile([128, 1], f32, tag="a_b")
            nc.gpsimd.tensor_scalar(out=a_b[:, :], in0=gb[:, 0:1],
                                    scalar1=1e-5, scalar2=None, op0=OP.max)
            qb = stats.tile([128, 1], f32, tag="qb")
            nc.gpsimd.tensor_scalar(out=qb[:, :], in0=gb[:, 1:2],
                                    scalar1=-QP, scalar2=QP * 1e5,
                                    op0=OP.mult, op1=OP.min)
            c0 = stats.tile([128, 1], f32, tag="c0")
            nc.gpsimd.tensor_tensor(out=c0[:, :], in0=a_b[:, :],
                                    in1=wb2[:, 0:1], op=OP.mult)
            cb = stats.tile([128, 1], f32, tag="cb")
            nc.gpsimd.tensor_scalar(out=cb[:, :], in0=c0[:, :],
                                    scalar1=1.0 / QP, scalar2=None,
                                    op0=OP.mult)

            # ---------- phase B: quantize (gpsimd) + GEMM + scaled output ----
            with (
                tc.tile_pool(name="aq", bufs=2) as aq_pool,
                tc.tile_pool(name="xqT", bufs=2) as xqT_pool,
                tc.tile_pool(name="outp", bufs=2) as out_pool,
                tc.tile_pool(name="psO", bufs=3, space="PSUM") as psO,
            ):
                pending = []   # (i, po, ot) awaiting psum->sbuf copy + dma

                def flush_one():
                    i0, po0, ot0 = pending.pop(0)
                    d0, c0_ = divmod(i0, SUB)
                    if i0 % 2 == 0:
                        nc.scalar.activation(out=ot0[:, :], in_=po0[:, :],
                                             func=AF.Copy, scale=cb[:, 0:1])
                    else:
                        nc.vector.tensor_scalar(out=ot0[:, :], in0=po0[:, :],
                                                scalar1=cb[:, 0:1],
                                                scalar2=None, op0=OP.mult)
                    # token = 4p + c: strided row DMA back to natural order
                    nc.sync.dma_start(
                        out=out_d[d0 * TPD:(d0 + 1) * TPD, :].rearrange(
                            "(p c) n -> p c n", p=128)[:, c0_, :],
                        in_=ot0[:, :])

                for i in range(NT):
                    aq = aq_pool.tile([128, DIN], f32, tag="aq")
                    nc.gpsimd.tensor_scalar(out=aq[:, :],
                                            in0=xnT_list[i][:, :],
                                            scalar1=qb[:, 0:1], scalar2=MAGIC,
                                            op0=OP.mult, op1=OP.add)
                    xqT = xqT_pool.tile([128, DIN], fp16, tag="xqT")
                    nc.gpsimd.tensor_scalar(out=xqT[:, :], in0=aq[:, :],
                                            scalar1=MAGIC, scalar2=None,
                                            op0=OP.subtract)

                    po = psO.tile([128, DOUT], f32, tag="po")
                    for j in range(KT):
                        for h in range(NH):
                            nc.tensor.matmul(
                                po[:, h * 512:(h + 1) * 512],
                                lhsT=xqT[:, j * 128:(j + 1) * 128],
                                rhs=wq_list[j][:, h * 512:(h + 1) * 512],
                                start=(j == 0), stop=(j == KT - 1))
                    ot = out_pool.tile([128, DOUT], fp16, tag="ot")
                    pending.append((i, po, ot))
                    if len(pending) > 1:
                        flush_one()
                while pending:
                    flush_one()

    nc.compile()
    return nc


def _get_nc(apply_nw: bool):
    key = ("nc", apply_nw)
    if key not in _CACHE:
        _CACHE[key] = _build(apply_nw)
    return _CACHE[key]


def _run(x, weight, norm_weight, trace=False):
    from concourse import bass_utils

    x = np.asarray(x)
    weight = np.ascontiguousarray(np.asarray(weight, dtype=np.float32))
    norm_weight = np.asarray(norm_weight, dtype=np.float32)

    apply_nw = not bool(np.all(norm_weight == 1.0))
    nc = _get_nc(apply_nw)

    xf = np.ascontiguousarray(x.reshape(TOK, DIN).astype(np.float16))
    wt = np.ascontiguousarray(weight.T)          # [DIN, DOUT]
    in_maps = []
    for c in range(N_CORES):
        m = {"x": np.ascontiguousarray(xf[c * TOK_C:(c + 1) * TOK_C]),
             "wt": wt}
        if apply_nw:
            m["nw"] = np.ascontiguousarray(
                np.broadcast_to(norm_weight.reshape(1, DIN),
                                (128, DIN)).astype(np.float16))
        in_maps.append(m)

    res = bass_utils.run_bass_kernel_spmd(
        nc, in_maps, core_ids=list(range(N_CORES)), trace=trace)

    out = np.empty((TOK, DOUT), dtype=np.float32)
    for c in range(N_CORES):
        out[c * TOK_C:(c + 1) * TOK_C] = res.results[c]["out"].astype(np.float32)
    return out.reshape(B, S, DOUT), res


def kernel(x, weight, norm_weight):
    out, _ = _run(x, weight, norm_weight, trace=False)
    return out


# revision 20
# speedup vs baseline: 2.0420x; 2.0420x over previous
"""BitLinear (RMSNorm + per-tensor 8-bit act quant + ternary weight quant + matmul)
as a distributed Bass/Tile kernel on 8 TRN2 NeuronCores.

Sharding: data-parallel over tokens (B*S = 32768 -> 4096 tokens/core).
Every core loads the full (host-pre-transposed) weight and computes
w_scale redundantly; the only collective is an AllGather of per-core
(max|xn|, -1/max|xn|) pairs (gathering the negated inverse avoids any
post-collective reciprocal: max_r(-1/m_r) == -1/max_r m_r).

v5 pipeline (engine-disciplined, strict-FIFO-safe):
  * x cast to fp16 on host.  Tokens are interleaved 4-per-partition
    ("(p c) k" instead of "(c p) k") so every DMA partition line is a
    contiguous 8KB run -- the 2KB-line layout measured only ~178 GB/s.
    The permutation is carried consistently through stats, the fused
    transpose, the GEMM and the output DMA.
  * Phase A: sumsq on scalar (Square+accum; tensor_tensor_reduce
    crashes real HW), per-chunk absmax on DVE.  Collective launches as
    soon as stats finish (~40us).
  * rms fused into the PE transpose via matmul against diag(rms).
  * Weight DMA interleaved with x DMA; |w| accumulation on DVE; weight
    quantization scalar+DVE, in place.
  * gpsimd owns diag builds, the collective path, and the entire
    phase-B quantize (scale+magic-add, magic-sub) so scalar/DVE only
    carry the PSUM->SBUF copies once the GEMM is running.
  * Output fp16, host upcasts.

Numerical core: x_q in [-127,127] integers, w_q in {-1,0,1}; fp16
matmul with fp32 PSUM accumulation is exact; rounding via the fp32
magic constant (1.5*2**23) matches jnp.round.
"""

import numpy as np

# ---- problem constants (hardcoded per contract) ----
B, S, DIN, DOUT = 4, 8192, 1024, 1024
N_CORES = 8
TOK = B * S                    # 32768 tokens
TOK_C = TOK // N_CORES         # 4096 tokens per core
TPD = 512                      # tokens per DMA chunk (4 per partition)
ND = TOK_C // TPD              # 8 DMA chunks per core
SUB = TPD // 128               # 4 sub-tiles per chunk (token = 4p + c)
NT = TOK_C // 128              # 32 sub-tiles
KT = DIN // 128                # 8 contraction tiles
NH = DOUT // 512               # 2 psum halves of the output row
EPS = 1e-6
QP = 127.0
MAGIC = 12582912.0             # 1.5 * 2**23: fp32 RNE round-to-int trick

_CACHE = {}


def _build(apply_nw: bool):
    import concourse.bass as bass
    import concourse.bacc as bacc
    import concourse.mybir as mybir
    import concourse.bass_isa as bass_isa
    from concourse import tile, masks

    f32 = mybir.dt.float32
    fp16 = mybir.dt.float16
    AF = mybir.ActivationFunctionType
    OP = mybir.AluOpType
    AX = mybir.AxisListType
    RED = bass_isa.ReduceOp

    nc = bacc.Bacc("TRN2", target_bir_lowering=False, debug=False,
                   num_devices=N_CORES)

    x_d = nc.dram_tensor("x", [TOK_C, DIN], fp16, kind="ExternalInput")
    wt_d = nc.dram_tensor("wt", [DIN, DOUT], f32, kind="ExternalInput")
    if apply_nw:
        nw_d = nc.dram_tensor("nw", [128, DIN], fp16, kind="ExternalInput")
    out_d = nc.dram_tensor("out", [TOK_C, DOUT], fp16, kind="ExternalOutput")

    with tile.TileContext(nc) as tc:
        with (
            tc.tile_pool(name="const", bufs=1) as const_pool,
            tc.tile_pool(name="stats", bufs=1) as stats,
            tc.tile_pool(name="xs", bufs=ND) as x_pool,
            tc.tile_pool(name="xnT", bufs=NT) as xnT_pool,
            tc.tile_pool(name="wts", bufs=KT) as wt_pool,
            tc.tile_pool(name="wqs", bufs=KT) as wq_pool,
            tc.tile_pool(name="sqscr", bufs=2) as sq_pool,
            tc.tile_pool(name="diag", bufs=NT) as diag_pool,
            tc.tile_pool(name="dram", bufs=1, space="DRAM") as dram_pool,
        ):
            ident_hf = const_pool.tile([128, 128], fp16, tag="ident_hf")
            masks.make_identity(nc, ident_hf[:, :])

            sumsq = stats.tile([128, NT], f32, tag="sumsq")
            amax = stats.tile([128, NT], f32, tag="amax")
            rms = stats.tile([128, NT], f32, tag="rms")
            wsum = stats.tile([128, KT], f32, tag="wsum")

            xnT_list = []
            wq_list = []
            dg_tiles = []
            wt_tiles = []

            if apply_nw:
                nwb = const_pool.tile([128, DIN], fp16, tag="nwb")
                nc.sync.dma_start(out=nwb[:, :], in_=nw_d[:, :])

            # ---------- x load (8KB lines) + stats; wt DMA interleaved ------
            x_tiles = []
            for d in range(ND):
                xt = x_pool.tile([128, SUB, DIN], fp16, tag="xt")
                nc.sync.dma_start(
                    out=xt[:, :, :],
                    in_=x_d[d * TPD:(d + 1) * TPD, :].rearrange(
                        "(p c) k -> p c k", p=128))
                if d >= 1:   # interleave weight loads behind the x stream
                    j = d - 1
                    wtt = wt_pool.tile([128, DOUT], f32, tag="wt")
                    nc.sync.dma_start(out=wtt[:, :],
                                      in_=wt_d[j * 128:(j + 1) * 128, :])
                    wt_tiles.append(wtt)
                if apply_nw:
                    xh = x_pool.tile([128, SUB, DIN], fp16, tag="xh")
                    for c in range(SUB):
                        nc.vector.tensor_tensor(out=xh[:, c, :],
                                                in0=xt[:, c, :],
                                                in1=nwb[:, :], op=OP.mult)
                else:
                    xh = xt
                x_tiles.append(xh)
                for c in range(SUB):
                    i = d * SUB + c
                    scr = sq_pool.tile([128, DIN], fp16, tag="sqa")
                    nc.scalar.activation(
                        out=scr[:, :], in_=xt[:, c, :], func=AF.Square,
                        accum_out=sumsq[:, i:i + 1])
                # per-chunk absmax: [128, SUB, DIN] -> [128, SUB] in one op
                sl = slice(d * SUB, (d + 1) * SUB)
                nc.vector.tensor_reduce(
                    out=amax[:, sl], in_=xh[:, :, :],
                    axis=AX.X, op=OP.max, apply_absolute_value=True)
                m2 = stats.tile([128, SUB], f32, tag="m2", name=f"m2_{d}")
                nc.vector.tensor_scalar(out=m2[:, :], in0=sumsq[:, sl],
                                        scalar1=1.0 / DIN, scalar2=EPS,
                                        op0=OP.mult, op1=OP.add)
                r2 = stats.tile([128, SUB], f32, tag="r2", name=f"r2_{d}")
                nc.vector.reciprocal(r2[:, :], m2[:, :])
                nc.scalar.activation(out=rms[:, sl], in_=r2[:, :],
                                     func=AF.Sqrt)
                # diag(rms) builds (DVE: tiny, ~0.08us each)
                for c in range(SUB):
                    i = d * SUB + c
                    dg = diag_pool.tile([128, 128], fp16, tag="dg",
                                        name=f"dg_{i}")
                    nc.vector.tensor_scalar(out=dg[:, :], in0=ident_hf[:, :],
                                            scalar1=rms[:, i:i + 1],
                                            scalar2=None, op0=OP.mult)
                    dg_tiles.append(dg)
            # last weight tile
            wtt = wt_pool.tile([128, DOUT], f32, tag="wt")
            nc.sync.dma_start(out=wtt[:, :], in_=wt_d[(KT - 1) * 128:, :])
            wt_tiles.append(wtt)

            # ---------- local |xn| max -> collective (no PE involved) --------
            axn = stats.tile([128, NT], f32, tag="axn")
            nc.vector.tensor_tensor(out=axn[:, :], in0=amax[:, :],
                                    in1=rms[:, :], op=OP.mult)
            axn2 = stats.tile([128, NT], f32, tag="axn2")
            nc.vector.tensor_scalar(out=axn2[:, :], in0=axn[:, :],
                                    scalar1=1e4, scalar2=None, op0=OP.min)
            lmax = stats.tile([128, 1], f32, tag="lmax")
            nc.vector.tensor_reduce(out=lmax[:, :], in_=axn2[:, :],
                                    axis=AX.X, op=OP.max)
            # partition-max via a tiny DMA round-trip (gpsimd partition ops
            # measured ~10-20us on HW; this path is ~2us)
            lrow_d = dram_pool.tile([1, 128], f32, tag="lrow_d")
            nc.sync.dma_start(out=lrow_d[:, :], in_=lmax[:, :])
            lrow = stats.tile([1, 128], f32, tag="lrow")
            nc.sync.dma_start(out=lrow[:, :], in_=lrow_d[:, :])
            gm = stats.tile([1, 1], f32, tag="gm")
            nc.vector.tensor_reduce(out=gm[:, :], in_=lrow[:, :],
                                    axis=AX.X, op=OP.max)
            ginv = stats.tile([1, 1], f32, tag="ginv")
            nc.vector.reciprocal(ginv[:, :], gm[:, :])
            pk = stats.tile([1, 2], f32, tag="pk")
            nc.vector.tensor_copy(pk[:, 0:1], gm[:, :])
            nc.vector.tensor_scalar(out=pk[:, 1:2], in0=ginv[:, :],
                                    scalar1=-1.0, scalar2=None, op0=OP.mult)

            cc_in = dram_pool.tile([1, 2], f32, tag="cc_in")
            cc_out = dram_pool.tile([1, 2 * N_CORES], f32, tag="cc_out")
            nc.sync.dma_start(out=cc_in[:, :], in_=pk[:, :])
            nc.gpsimd.collective_compute(
                "AllGather", OP.bypass,
                replica_groups=[list(range(N_CORES))],
                ins=[cc_in[:, :].opt()],
                outs=[cc_out[:, :].opt()],
            )

            # ---------- w_scale (|w| accumulation on scalar) -----------------
            for j in range(KT):
                scr = sq_pool.tile([128, DOUT], fp16, tag="sqa")
                nc.scalar.activation(out=scr[:, :], in_=wt_tiles[j][:, :],
                                     func=AF.Abs,
                                     accum_out=wsum[:, j:j + 1])
            wred = stats.tile([128, 1], f32, tag="wred")
            nc.vector.tensor_reduce(out=wred[:, :], in_=wsum[:, :],
                                    axis=AX.X, op=OP.add)
            wrow_d = dram_pool.tile([1, 128], f32, tag="wrow_d")
            nc.sync.dma_start(out=wrow_d[:, :], in_=wred[:, :])
            wrow = stats.tile([1, 128], f32, tag="wrow")
            nc.sync.dma_start(out=wrow[:, :], in_=wrow_d[:, :])
            wtot = stats.tile([1, 1], f32, tag="wtot")
            nc.vector.tensor_reduce(out=wtot[:, :], in_=wrow[:, :],
                                    axis=AX.X, op=OP.add)
            wsc = stats.tile([1, 1], f32, tag="wsc")
            nc.vector.tensor_scalar(out=wsc[:, :], in0=wtot[:, :],
                                    scalar1=1.0 / (DIN * DOUT),
                                    scalar2=1e-4, op0=OP.mult, op1=OP.max)
            inv_ws = stats.tile([1, 1], f32, tag="inv_ws")
            nc.vector.reciprocal(inv_ws[:, :], wsc[:, :])
            wpk = stats.tile([1, 2], f32, tag="wpk")
            nc.vector.tensor_copy(wpk[:, 0:1], wsc[:, :])
            nc.vector.tensor_copy(wpk[:, 1:2], inv_ws[:, :])
            wb_d = dram_pool.tile([1, 2], f32, tag="wb_d")
            nc.sync.dma_start(out=wb_d[:, :], in_=wpk[:, :])
            wb2 = stats.tile([128, 2], f32, tag="wb2")
            nc.sync.dma_start(out=wb2[:, :],
                              in_=wb_d[0:1, :].broadcast_to([128, 2]))

            # weight quantization: scalar magic-round, DVE clip (in place)
            for j in range(KT):
                nc.scalar.activation(out=wt_tiles[j][:, :],
                                     in_=wt_tiles[j][:, :], func=AF.Copy,
                                     scale=wb2[:, 1:2], bias=MAGIC)
                nc.vector.tensor_scalar(out=wt_tiles[j][:, :],
                                        in0=wt_tiles[j][:, :],
                                        scalar1=MAGIC, scalar2=1.0,
                                        op0=OP.subtract, op1=OP.min)
                wq = wq_pool.tile([128, DOUT], fp16, tag="wq")
                nc.vector.tensor_scalar(out=wq[:, :], in0=wt_tiles[j][:, :],
                                        scalar1=-1.0, scalar2=None,
                                        op0=OP.max)
                wq_list.append(wq)

            # ---------- transposes: diag(rms)-fused, fill the cc bubble ------
            with tc.tile_pool(name="psA", bufs=3, space="PSUM") as psA:
                for i in range(NT):
                    d, c = divmod(i, SUB)
                    pA = psA.tile([128, DIN], f32, tag="pA")
                    for j in range(KT):
                        nc.tensor.matmul(
                            pA[:, j * 128:(j + 1) * 128],
                            lhsT=x_tiles[d][:, c, j * 128:(j + 1) * 128],
                            rhs=dg_tiles[i][:, :], start=True, stop=True)
                    xnT = xnT_pool.tile([128, DIN], fp16, tag="xnT",
                                        name=f"xnT_{i}")
                    xnT_list.append(xnT)
                    # psum f32 -> sbuf fp16: ~17 on ACT, ~15 on DVE
                    if i % 2 == 0 or i == 31:
                        nc.scalar.activation(out=xnT[:, :], in_=pA[:, :],
                                             func=AF.Copy)
                    else:
                        nc.vector.tensor_copy(xnT[:, :], pA[:, :])

            # ---------- collective read-back + scales (DVE + DMA bcast) ------
            cc_sb = stats.tile([1, 2 * N_CORES], f32, tag="cc_sb")
            nc.sync.dma_start(out=cc_sb[:, :], in_=cc_out[:, :])
            # view [1, 2, 8]: reduce over ranks for both columns at once:
            # col0 -> max_r gmax_r = a;  col1 -> max_r(-1/g_r) = -1/a
            mx = stats.tile([1, 2], f32, tag="mx")
            nc.vector.tensor_reduce(
                out=mx[:, :],
                in_=cc_sb[0:1, :].rearrange("a (r c) -> a c r", c=2),
                axis=AX.X, op=OP.max)
            a1 = stats.tile([1, 1], f32, tag="a1")
            nc.vector.tensor_scalar(out=a1[:, :], in0=mx[:, 0:1],
                                    scalar1=1e-5, scalar2=None, op0=OP.max)
            qcb = stats.tile([1, 2], f32, tag="qcb")
            # qb = 127/max(a, 1e-5)
            nc.vector.tensor_scalar(out=qcb[:, 0:1], in0=mx[:, 1:2],
                                    scalar1=-QP, scalar2=QP * 1e5,
                                    op0=OP.mult, op1=OP.min)
            # cb = a * w_scale / 127
            cbs = stats.tile([1, 1], f32, tag="cbs")
            nc.vector.tensor_tensor(out=cbs[:, :], in0=a1[:, :],
                                    in1=wsc[:, :], op=OP.mult)
            nc.vector.tensor_scalar(out=qcb[:, 1:2], in0=cbs[:, :],
                                    scalar1=1.0 / QP, scalar2=None,
                                    op0=OP.mult)
            qcb_d = dram_pool.tile([1, 2], f32, tag="qcb_d")
            nc.sync.dma_start(out=qcb_d[:, :], in_=qcb[:, :])
            qcb2 = stats.tile([128, 2], f32, tag="qcb2")
            nc.sync.dma_start(out=qcb2[:, :],
                              in_=qcb_d[0:1, :].broadcast_to([128, 2]))
            qb = qcb2[:, 0:1]
            cb = qcb2[:, 1:2]

            # ---------- phase B: quantize (gpsimd) + GEMM + scaled output ----
            with (
                tc.tile_pool(name="aq", bufs=2) as aq_pool,
                tc.tile_pool(name="xqT", bufs=2) as xqT_pool,
                tc.tile_pool(name="outp", bufs=2) as out_pool,
                tc.tile_pool(name="psO", bufs=3, space="PSUM") as psO,
            ):
                pending = []   # (i, po, ot) awaiting psum->sbuf copy + dma

                def flush_one():
                    i0, po0, ot0 = pending.pop(0)
                    d0, c0_ = divmod(i0, SUB)
                    if i0 % 2 == 0:
                        nc.scalar.activation(out=ot0[:, :], in_=po0[:, :],
                                             func=AF.Copy, scale=cb[:, 0:1])
                    else:
                        nc.vector.tensor_scalar(out=ot0[:, :], in0=po0[:, :],
                                                scalar1=cb[:, 0:1],
                                                scalar2=None, op0=OP.mult)
                    # token = 4p + c: strided row DMA back to natural order
                    nc.sync.dma_start(
                        out=out_d[d0 * TPD:(d0 + 1) * TPD, :].rearrange(
                            "(p c) n -> p c n", p=128)[:, c0_, :],
                        in_=ot0[:, :])

                for i in range(NT):
                    aq = aq_pool.tile([128, DIN], f32, tag="aq")
                    nc.gpsimd.tensor_scalar(out=aq[:, :],
                                            in0=xnT_list[i][:, :],
                                            scalar1=qb[:, 0:1], scalar2=MAGIC,
                                            op0=OP.mult, op1=OP.add)
                    xqT = xqT_pool.tile([128, DIN], fp16, tag="xqT")
                    nc.vector.tensor_scalar(out=xqT[:, :], in0=aq[:, :],
                                            scalar1=MAGIC, scalar2=None,
                                            op0=OP.subtract)

                    po = psO.tile([128, DOUT], f32, tag="po")
                    for j in range(KT):
                        for h in range(NH):
                            nc.tensor.matmul(
                                po[:, h * 512:(h + 1) * 512],
                                lhsT=xqT[:, j * 128:(j + 1) * 128],
                                rhs=wq_list[j][:, h * 512:(h + 1) * 512],
                                start=(j == 0), stop=(j == KT - 1))
                    ot = out_pool.tile([128, DOUT], fp16, tag="ot")
                    pending.append((i, po, ot))
                    if len(pending) > 1:
                        flush_one()
                while pending:
                    flush_one()

    nc.compile()
    return nc


def _get_nc(apply_nw: bool):
    key = ("nc", apply_nw)
    if key not in _CACHE:
        _CACHE[key] = _build(apply_nw)
    return _CACHE[key]


def _run(x, weight, norm_weight, trace=False):
    from concourse import bass_utils

    x = np.asarray(x)
    weight = np.ascontiguousarray(np.asarray(weight, dtype=np.float32))
    norm_weight = np.asarray(norm_weight, dtype=np.float32)

    apply_nw = not bool(np.all(norm_weight == 1.0))
    nc = _get_nc(apply_nw)

    xf = np.ascontiguousarray(x.reshape(TOK, DIN).astype(np.float16))
    wt = np.ascontiguousarray(weight.T)          # [DIN, DOUT]
    in_maps = []
    for c in range(N_CORES):
        m = {"x": np.ascontiguousarray(xf[c * TOK_C:(c + 1) * TOK_C]),
             "wt": wt}
        if apply_nw:
            m["nw"] = np.ascontiguousarray(
                np.broadcast_to(norm_weight.reshape(1, DIN),
                                (128, DIN)).astype(np.float16))
        in_maps.append(m)

    res = bass_utils.run_bass_kernel_spmd(
        nc, in_maps, core_ids=list(range(N_CORES)), trace=trace)

    out = np.empty((TOK, DOUT), dtype=np.float32)
    for c in range(N_CORES):
        out[c * TOK_C:(c + 1) * TOK_C] = res.results[c]["out"].astype(np.float32)
    return out.reshape(B, S, DOUT), res


def kernel(x, weight, norm_weight):
    out, _ = _run(x, weight, norm_weight, trace=False)
    return out


# revision 27
# speedup vs baseline: 2.7779x; 1.3604x over previous
"""BitLinear (RMSNorm + per-tensor 8-bit act quant + ternary weight quant + matmul)
as a distributed Bass/Tile kernel on 8 TRN2 NeuronCores.

Sharding: data-parallel over tokens (B*S = 32768 -> 4096 tokens/core).
Every core loads the full (host-pre-transposed) weight and computes
w_scale redundantly; the only collective is an AllGather of per-core
(max|xn|, -1/max|xn|) pairs (gathering the negated inverse avoids any
post-collective reciprocal: max_r(-1/m_r) == -1/max_r m_r).

v5 pipeline (engine-disciplined, strict-FIFO-safe):
  * x cast to fp16 on host.  Tokens are interleaved 4-per-partition
    ("(p c) k" instead of "(c p) k") so every DMA partition line is a
    contiguous 8KB run -- the 2KB-line layout measured only ~178 GB/s.
    The permutation is carried consistently through stats, the fused
    transpose, the GEMM and the output DMA.
  * Phase A: sumsq on scalar (Square+accum; tensor_tensor_reduce
    crashes real HW), per-chunk absmax on DVE.  Collective launches as
    soon as stats finish (~40us).
  * rms fused into the PE transpose via matmul against diag(rms).
  * Weight DMA interleaved with x DMA; |w| accumulation on DVE; weight
    quantization scalar+DVE, in place.
  * gpsimd owns diag builds, the collective path, and the entire
    phase-B quantize (scale+magic-add, magic-sub) so scalar/DVE only
    carry the PSUM->SBUF copies once the GEMM is running.
  * Output fp16, host upcasts.

Numerical core: x_q in [-127,127] integers, w_q in {-1,0,1}; fp16
matmul with fp32 PSUM accumulation is exact; rounding via the fp32
magic constant (1.5*2**23) matches jnp.round.
"""

import numpy as np

# ---- problem constants (hardcoded per contract) ----
B, S, DIN, DOUT = 4, 8192, 1024, 1024
N_CORES = 8
TOK = B * S                    # 32768 tokens
TOK_C = TOK // N_CORES         # 4096 tokens per core
TPD = 512                      # tokens per DMA chunk (4 per partition)
ND = TOK_C // TPD              # 8 DMA chunks per core
SUB = TPD // 128               # 4 sub-tiles per chunk (token = 4p + c)
NT = TOK_C // 128              # 32 sub-tiles
KT = DIN // 128                # 8 contraction tiles
NH = DOUT // 512               # 2 psum halves of the output row
EPS = 1e-6
QP = 127.0
MAGIC = 12582912.0             # 1.5 * 2**23: fp32 RNE round-to-int trick

_CACHE = {}


def _build(apply_nw: bool):
    import concourse.bass as bass
    import concourse.bacc as bacc
    import concourse.mybir as mybir
    import concourse.bass_isa as bass_isa
    from concourse import tile, masks

    f32 = mybir.dt.float32
    fp16 = mybir.dt.float16
    AF = mybir.ActivationFunctionType
    OP = mybir.AluOpType
    AX = mybir.AxisListType
    RED = bass_isa.ReduceOp

    nc = bacc.Bacc("TRN2", target_bir_lowering=False, debug=False,
                   num_devices=N_CORES)

    x_d = nc.dram_tensor("x", [TOK_C, DIN], fp16, kind="ExternalInput")
    wt_d = nc.dram_tensor("wt", [DIN, DOUT], f32, kind="ExternalInput")
    if apply_nw:
        nw_d = nc.dram_tensor("nw", [128, DIN], fp16, kind="ExternalInput")
    out_d = nc.dram_tensor("out", [TOK_C, DOUT], fp16, kind="ExternalOutput")

    with tile.TileContext(nc) as tc:
        with (
            tc.tile_pool(name="const", bufs=1) as const_pool,
            tc.tile_pool(name="psS", bufs=1, space="PSUM") as psS,
            tc.tile_pool(name="stats", bufs=1) as stats,
            tc.tile_pool(name="xs", bufs=ND) as x_pool,
            tc.tile_pool(name="xnT", bufs=NT) as xnT_pool,
            tc.tile_pool(name="wts", bufs=KT) as wt_pool,
            tc.tile_pool(name="wqs", bufs=KT) as wq_pool,
            tc.tile_pool(name="sqscr", bufs=2) as sq_pool,
            tc.tile_pool(name="diag", bufs=NT) as diag_pool,
            tc.tile_pool(name="dram", bufs=1, space="DRAM") as dram_pool,
        ):
            ident_hf = const_pool.tile([128, 128], fp16, tag="ident_hf")
            masks.make_identity(nc, ident_hf[:, :])
            ident_f32 = const_pool.tile([128, 128], f32, tag="ident_f32")
            masks.make_identity(nc, ident_f32[:, :])
            ones_row = const_pool.tile([1, 128], f32, tag="ones_row")
            nc.gpsimd.memset(ones_row[:, :], 1.0)

            sumsq = stats.tile([128, NT], f32, tag="sumsq")
            amax = stats.tile([128, NT], f32, tag="amax")
            rms = stats.tile([128, NT], f32, tag="rms")
            wsum = stats.tile([128, KT], f32, tag="wsum")

            def part_reduce(vec128, op, tag):
                """[128,1] fp32 -> [1,1] via PE transpose + DVE reduce."""
                pt = psS.tile([1, 128], f32, tag="pt", name="pt_" + tag)
                nc.tensor.transpose(pt[:, :], vec128, ident_f32[:, :])
                sb = stats.tile([1, 128], f32, tag=tag + "_row",
                                name=tag + "_row")
                nc.vector.tensor_copy(sb[:, :], pt[:, :])
                r = stats.tile([1, 1], f32, tag=tag, name=tag)
                nc.vector.tensor_reduce(out=r[:, :], in_=sb[:, :],
                                        axis=AX.X, op=op)
                return r

            def bcast_scalar(src, tag):
                """[1,1] fp32 -> [128,1] via ones-matmul on PE."""
                pb = psS.tile([128, 1], f32, tag="pb", name="pb_" + tag)
                nc.tensor.matmul(pb[:, :], lhsT=ones_row[:, :], rhs=src,
                                 start=True, stop=True)
                dst = stats.tile([128, 1], f32, tag=tag, name=tag)
                nc.vector.tensor_copy(dst[:, :], pb[:, :])
                return dst

            xnT_list = []
            wq_list = []
            dg_tiles = []
            wt_tiles = []

            if apply_nw:
                nwb = const_pool.tile([128, DIN], fp16, tag="nwb")
                nc.sync.dma_start(out=nwb[:, :], in_=nw_d[:, :])

            # ---------- x load (8KB lines) + stats; wt DMA interleaved ------
            x_tiles = []
            for d in range(ND):
                xt = x_pool.tile([128, SUB, DIN], fp16, tag="xt")
                nc.sync.dma_start(
                    out=xt[:, :, :],
                    in_=x_d[d * TPD:(d + 1) * TPD, :].rearrange(
                        "(p c) k -> p c k", p=128))
                if d >= 1:   # interleave weight loads behind the x stream
                    j = d - 1
                    wtt = wt_pool.tile([128, DOUT], f32, tag="wt")
                    nc.sync.dma_start(out=wtt[:, :],
                                      in_=wt_d[j * 128:(j + 1) * 128, :])
                    wt_tiles.append(wtt)
                if apply_nw:
                    xh = x_pool.tile([128, SUB, DIN], fp16, tag="xh")
                    for c in range(SUB):
                        nc.vector.tensor_tensor(out=xh[:, c, :],
                                                in0=xt[:, c, :],
                                                in1=nwb[:, :], op=OP.mult)
                else:
                    xh = xt
                x_tiles.append(xh)
                for c in range(SUB):
                    i = d * SUB + c
                    scr = sq_pool.tile([128, DIN], fp16, tag="sqa")
                    nc.scalar.activation(
                        out=scr[:, :], in_=xt[:, c, :], func=AF.Square,
                        accum_out=sumsq[:, i:i + 1])
                # per-chunk absmax: [128, SUB, DIN] -> [128, SUB] in one op
                sl = slice(d * SUB, (d + 1) * SUB)
                nc.vector.tensor_reduce(
                    out=amax[:, sl], in_=xh[:, :, :],
                    axis=AX.X, op=OP.max, apply_absolute_value=True)
                m2 = stats.tile([128, SUB], f32, tag="m2", name=f"m2_{d}")
                nc.vector.tensor_scalar(out=m2[:, :], in0=sumsq[:, sl],
                                        scalar1=1.0 / DIN, scalar2=EPS,
                                        op0=OP.mult, op1=OP.add)
                r2 = stats.tile([128, SUB], f32, tag="r2", name=f"r2_{d}")
                nc.vector.reciprocal(r2[:, :], m2[:, :])
                nc.scalar.activation(out=rms[:, sl], in_=r2[:, :],
                                     func=AF.Sqrt)
                # diag(rms) builds (DVE: tiny, ~0.08us each)
                for c in range(SUB):
                    i = d * SUB + c
                    dg = diag_pool.tile([128, 128], fp16, tag="dg",
                                        name=f"dg_{i}")
                    nc.vector.tensor_scalar(out=dg[:, :], in0=ident_hf[:, :],
                                            scalar1=rms[:, i:i + 1],
                                            scalar2=None, op0=OP.mult)
                    dg_tiles.append(dg)
            # last weight tile
            wtt = wt_pool.tile([128, DOUT], f32, tag="wt")
            nc.sync.dma_start(out=wtt[:, :], in_=wt_d[(KT - 1) * 128:, :])
            wt_tiles.append(wtt)

            # ---------- local |xn| max -> collective (no PE involved) --------
            axn = stats.tile([128, NT], f32, tag="axn")
            nc.vector.tensor_tensor(out=axn[:, :], in0=amax[:, :],
                                    in1=rms[:, :], op=OP.mult)
            axn2 = stats.tile([128, NT], f32, tag="axn2")
            nc.vector.tensor_scalar(out=axn2[:, :], in0=axn[:, :],
                                    scalar1=1e4, scalar2=None, op0=OP.min)
            lmax = stats.tile([128, 1], f32, tag="lmax")
            nc.vector.tensor_reduce(out=lmax[:, :], in_=axn2[:, :],
                                    axis=AX.X, op=OP.max)
            # partition-max via PE transpose (DMA round-trips cost ~8us/hop,
            # gpsimd partition ops 10-20us; this is ~1us and PE is idle)
            gm = part_reduce(lmax[:, :], OP.max, "gm")
            ginv = stats.tile([1, 1], f32, tag="ginv")
            nc.vector.reciprocal(ginv[:, :], gm[:, :])
            pk = stats.tile([1, 2], f32, tag="pk")
            nc.vector.tensor_copy(pk[:, 0:1], gm[:, :])
            nc.vector.tensor_scalar(out=pk[:, 1:2], in0=ginv[:, :],
                                    scalar1=-1.0, scalar2=None, op0=OP.mult)

            cc_in = dram_pool.tile([1, 2], f32, tag="cc_in")
            cc_out = dram_pool.tile([1, 2 * N_CORES], f32, tag="cc_out")
            nc.sync.dma_start(out=cc_in[:, :], in_=pk[:, :])
            nc.gpsimd.collective_compute(
                "AllGather", OP.bypass,
                replica_groups=[list(range(N_CORES))],
                ins=[cc_in[:, :].opt()],
                outs=[cc_out[:, :].opt()],
            )

            # ---------- w_scale (|w| accumulation on scalar) -----------------
            for j in range(KT):
                scr = sq_pool.tile([128, DOUT], fp16, tag="sqa")
                nc.scalar.activation(out=scr[:, :], in_=wt_tiles[j][:, :],
                                     func=AF.Abs,
                                     accum_out=wsum[:, j:j + 1])
            wred = stats.tile([128, 1], f32, tag="wred")
            nc.vector.tensor_reduce(out=wred[:, :], in_=wsum[:, :],
                                    axis=AX.X, op=OP.add)
            wtot = part_reduce(wred[:, :], OP.add, "wtot")
            wsc = stats.tile([1, 1], f32, tag="wsc")
            nc.vector.tensor_scalar(out=wsc[:, :], in0=wtot[:, :],
                                    scalar1=1.0 / (DIN * DOUT),
                                    scalar2=1e-4, op0=OP.mult, op1=OP.max)
            inv_ws = stats.tile([1, 1], f32, tag="inv_ws")
            nc.vector.reciprocal(inv_ws[:, :], wsc[:, :])
            inv_ws_b = bcast_scalar(inv_ws[:, :], "inv_ws_b")

            # weight quantization: scalar magic-round, DVE clip (in place)
            for j in range(KT):
                nc.scalar.activation(out=wt_tiles[j][:, :],
                                     in_=wt_tiles[j][:, :], func=AF.Copy,
                                     scale=inv_ws_b[:, 0:1], bias=MAGIC)
                nc.vector.tensor_scalar(out=wt_tiles[j][:, :],
                                        in0=wt_tiles[j][:, :],
                                        scalar1=MAGIC, scalar2=1.0,
                                        op0=OP.subtract, op1=OP.min)
                wq = wq_pool.tile([128, DOUT], fp16, tag="wq")
                nc.vector.tensor_scalar(out=wq[:, :], in0=wt_tiles[j][:, :],
                                        scalar1=-1.0, scalar2=None,
                                        op0=OP.max)
                wq_list.append(wq)

            # ---------- transposes: diag(rms)-fused, fill the cc bubble ------
            with tc.tile_pool(name="psA", bufs=3, space="PSUM") as psA:
                for i in range(NT):
                    d, c = divmod(i, SUB)
                    pA = psA.tile([128, DIN], f32, tag="pA")
                    for j in range(KT):
                        nc.tensor.matmul(
                            pA[:, j * 128:(j + 1) * 128],
                            lhsT=x_tiles[d][:, c, j * 128:(j + 1) * 128],
                            rhs=dg_tiles[i][:, :], start=True, stop=True)
                    xnT = xnT_pool.tile([128, DIN], fp16, tag="xnT",
                                        name=f"xnT_{i}")
                    xnT_list.append(xnT)
                    # psum f32 -> sbuf fp16: ~17 on ACT, ~15 on DVE
                    if i % 2 == 0 or i == 31:
                        nc.scalar.activation(out=xnT[:, :], in_=pA[:, :],
                                             func=AF.Copy)
                    else:
                        nc.vector.tensor_copy(xnT[:, :], pA[:, :])

            # ---------- collective read-back + scales (DVE + DMA bcast) ------
            cc_sb = stats.tile([1, 2 * N_CORES], f32, tag="cc_sb")
            nc.sync.dma_start(out=cc_sb[:, :], in_=cc_out[:, :])
            # view [1, 2, 8]: reduce over ranks for both columns at once:
            # col0 -> max_r gmax_r = a;  col1 -> max_r(-1/g_r) = -1/a
            mx = stats.tile([1, 2], f32, tag="mx")
            nc.vector.tensor_reduce(
                out=mx[:, :],
                in_=cc_sb[0:1, :].rearrange("a (r c) -> a c r", c=2),
                axis=AX.X, op=OP.max)
            a1 = stats.tile([1, 1], f32, tag="a1")
            nc.vector.tensor_scalar(out=a1[:, :], in0=mx[:, 0:1],
                                    scalar1=1e-5, scalar2=None, op0=OP.max)
            # qb = 127/max(a, 1e-5)
            qbs = stats.tile([1, 1], f32, tag="qbs")
            nc.vector.tensor_scalar(out=qbs[:, :], in0=mx[:, 1:2],
                                    scalar1=-QP, scalar2=QP * 1e5,
                                    op0=OP.mult, op1=OP.min)
            # cb = a * w_scale / 127
            cbs = stats.tile([1, 1], f32, tag="cbs")
            nc.vector.tensor_tensor(out=cbs[:, :], in0=a1[:, :],
                                    in1=wsc[:, :], op=OP.mult)
            cbs2 = stats.tile([1, 1], f32, tag="cbs2")
            nc.vector.tensor_scalar(out=cbs2[:, :], in0=cbs[:, :],
                                    scalar1=1.0 / QP, scalar2=None,
                                    op0=OP.mult)
            qb = bcast_scalar(qbs[:, :], "qb")
            cb = bcast_scalar(cbs2[:, :], "cb")

            # ---------- phase B: quantize (gpsimd) + GEMM + scaled output ----
            with (
                tc.tile_pool(name="aq", bufs=2) as aq_pool,
                tc.tile_pool(name="xqT", bufs=2) as xqT_pool,
                tc.tile_pool(name="outp", bufs=2) as out_pool,
                tc.tile_pool(name="psO", bufs=3, space="PSUM") as psO,
            ):
                pending = []   # (i, po, ot) awaiting psum->sbuf copy + dma

                def flush_one():
                    i0, po0, ot0 = pending.pop(0)
                    d0, c0_ = divmod(i0, SUB)
                    if i0 % 2 == 0:
                        nc.scalar.activation(out=ot0[:, :], in_=po0[:, :],
                                             func=AF.Copy, scale=cb[:, 0:1])
                    else:
                        nc.vector.tensor_scalar(out=ot0[:, :], in0=po0[:, :],
                                                scalar1=cb[:, 0:1],
                                                scalar2=None, op0=OP.mult)
                    # token = 4p + c: strided row DMA back to natural order
                    nc.sync.dma_start(
                        out=out_d[d0 * TPD:(d0 + 1) * TPD, :].rearrange(
                            "(p c) n -> p c n", p=128)[:, c0_, :],
                        in_=ot0[:, :])

                for i in range(NT):
                    aq = aq_pool.tile([128, DIN], f32, tag="aq")
                    nc.gpsimd.tensor_scalar(out=aq[:, :],
                                            in0=xnT_list[i][:, :],
                                            scalar1=qb[:, 0:1], scalar2=MAGIC,
                                            op0=OP.mult, op1=OP.add)
                    xqT = xqT_pool.tile([128, DIN], fp16, tag="xqT")
                    nc.vector.tensor_scalar(out=xqT[:, :], in0=aq[:, :],
                                            scalar1=MAGIC, scalar2=None,
                                            op0=OP.subtract)

                    po = psO.tile([128, DOUT], f32, tag="po")
                    for j in range(KT):
                        for h in range(NH):
                            nc.tensor.matmul(
                                po[:, h * 512:(h + 1) * 512],
                                lhsT=xqT[:, j * 128:(j + 1) * 128],
                                rhs=wq_list[j][:, h * 512:(h + 1) * 512],
                                start=(j == 0), stop=(j == KT - 1))
                    ot = out_pool.tile([128, DOUT], fp16, tag="ot")
                    pending.append((i, po, ot))
                    if len(pending) > 1:
                        flush_one()
                while pending:
                    flush_one()

    nc.compile()
    return nc


def _get_nc(apply_nw: bool):
    key = ("nc", apply_nw)
    if key not in _CACHE:
        _CACHE[key] = _build(apply_nw)
    return _CACHE[key]


def _run(x, weight, norm_weight, trace=False):
    from concourse import bass_utils

    x = np.asarray(x)
    weight = np.ascontiguousarray(np.asarray(weight, dtype=np.float32))
    norm_weight = np.asarray(norm_weight, dtype=np.float32)

    apply_nw = not bool(np.all(norm_weight == 1.0))
    nc = _get_nc(apply_nw)

    xf = np.ascontiguousarray(x.reshape(TOK, DIN).astype(np.float16))
    wt = np.ascontiguousarray(weight.T)          # [DIN, DOUT]
    in_maps = []
    for c in range(N_CORES):
        m = {"x": np.ascontiguousarray(xf[c * TOK_C:(c + 1) * TOK_C]),
             "wt": wt}
        if apply_nw:
            m["nw"] = np.ascontiguousarray(
                np.broadcast_to(norm_weight.reshape(1, DIN),
                                (128, DIN)).astype(np.float16))
        in_maps.append(m)

    res = bass_utils.run_bass_kernel_spmd(
        nc, in_maps, core_ids=list(range(N_CORES)), trace=trace)

    out = np.empty((TOK, DOUT), dtype=np.float32)
    for c in range(N_CORES):
        out[c * TOK_C:(c + 1) * TOK_C] = res.results[c]["out"].astype(np.float32)
    return out.reshape(B, S, DOUT), res


def kernel(x, weight, norm_weight):
    out, _ = _run(x, weight, norm_weight, trace=False)
    return out
